# revision 4
# baseline (speedup 1.0000x reference)
"""Disentangled spatial attention TRN2 kernel.

Sharding: 8 cores = 2 batches x 4 head-groups (4 heads each).
Per core, everything is computed in "transposed activation" layout:
  qcat[h] (128, L):  rows 0:64 qt_h, rows 64:128 qs_h
  kcat[h] (128, L):  rows 0:64 k1_h = kt + lam_ts*ks,
                     rows 64:128 k2_h = lam_st*kt + lam_ss*ks
  scores^T chunk = kcat_chunk.T @ qcat  (K=128 contraction = both score
  matmuls of the reference fused into one)
  softmax row-sums ride along the PV matmul as 64 replicated "ones"
  columns of the v operand; normalization happens on the way to the
  transposed y layout that feeds the output projection.
Host-side: inputs are sharded/transposed, lam_* are folded into the
weight shards, per-core partial projections are summed per batch.
"""
import numpy as np
import concourse.bass as bass
import concourse.mybir as mybir
import concourse.tile as tile
from concourse.bass_utils import run_bass_kernel_spmd

F32 = mybir.dt.float32
F32R = mybir.dt.float32r
AF = mybir.ActivationFunctionType

B, L, E, H, D = 2, 2048, 1024, 16, 64
HPC = 4          # heads per core
NCORES = 8
LTB = 256        # L block for phase 1
NLTB = L // LTB  # 8
NCHUNK = L // 128  # 16 Lk chunks
EC = E // 128    # 8 E chunks


def _split_multi_waits(nc, max_waits=1):
    """walrus codegen allows only one sync wait per instruction; move extra
    waits onto standalone same-engine NoOps placed just before."""
    n_split = 0
    for f in nc.m.functions:
        for blk in f.blocks:
            insts = list(blk.instructions)
            out = []
            changed = False
            for inst in insts:
                si = inst.sync_info
                waits = list(si.on_wait) if si is not None and si.on_wait else []
                if len(waits) > max_waits:
                    keep = waits[-max_waits:]
                    extra = waits[:-max_waits]
                    for w in extra:
                        nop = mybir.InstNoOp(
                            name=f"{inst.name}-wsplit{n_split}",
                            engine=inst.engine,
                            ins=[], outs=[],
                            sync_info=mybir.SyncInfo(on_wait=[w], on_update=[]),
                        )
                        out.append(nop)
                        n_split += 1
                    inst.sync_info = mybir.SyncInfo(
                        on_wait=keep,
                        on_update=list(si.on_update) if si.on_update else [],
                    )
                    changed = True
                out.append(inst)
            if changed:
                blk.instructions = out
    return n_split


def _build():
    nc = bass.Bass()
    xtT = nc.declare_dram_parameter("xtT", [E, L], F32R, isOutput=False)
    xsT = nc.declare_dram_parameter("xsT", [E, L], F32R, isOutput=False)
    wq = nc.declare_dram_parameter("wq", [128, EC, HPC * D], F32R, isOutput=False)
    wqs = nc.declare_dram_parameter("wqs", [128, EC, HPC * D], F32R, isOutput=False)
    wk = nc.declare_dram_parameter("wk", [128, 2 * EC, 2 * HPC * D], F32R, isOutput=False)
    wv = nc.declare_dram_parameter("wv", [128, EC, HPC * D], F32R, isOutput=False)
    wc = nc.declare_dram_parameter("wc", [128, 2, E], F32R, isOutput=False)
    bq = nc.declare_dram_parameter("bq", [128, 2], F32, isOutput=False)
    bqs = nc.declare_dram_parameter("bqs", [128, 2], F32, isOutput=False)
    bk = nc.declare_dram_parameter("bk", [128, HPC], F32, isOutput=False)
    bv = nc.declare_dram_parameter("bv", [1, HPC * D], F32R, isOutput=False)
    bc = nc.declare_dram_parameter("bc", [1, E], F32R, isOutput=False)
    ones = nc.declare_dram_parameter("ones", [128, 128], F32R, isOutput=False)
    out = nc.declare_dram_parameter("out", [L, E], F32, isOutput=True)

    xtT_v = xtT.rearrange("(k p) l -> p k l", p=128)   # (128, 8, L)
    xsT_v = xsT.rearrange("(k p) l -> p k l", p=128)

    with tile.TileContext(nc) as tc:
        with tc.tile_pool(name="wpool", bufs=1) as wpool, \
             tc.tile_pool(name="persist", bufs=1) as pp:
            wq_sb = wpool.tile([128, EC, HPC * D], F32R)
            wqs_sb = wpool.tile([128, EC, HPC * D], F32R)
            wk_sb = wpool.tile([128, 2 * EC, 2 * HPC * D], F32R)
            wv_sb = wpool.tile([128, EC, HPC * D], F32R)
            bq_sb = wpool.tile([128, 2], F32)
            bqs_sb = wpool.tile([128, 2], F32)
            bk_sb = wpool.tile([128, HPC], F32)
            bv_sb = wpool.tile([1, HPC * D], F32R)
            ones_sb = wpool.tile([128, 128], F32R)
            nc.sync.dma_start(wq_sb[:], wq[:])
            nc.sync.dma_start(wqs_sb[:], wqs[:])
            nc.sync.dma_start(wk_sb[:], wk[:])
            nc.sync.dma_start(wv_sb[:], wv[:])
            nc.sync.dma_start(bq_sb[:], bq[:])
            nc.sync.dma_start(bqs_sb[:], bqs[:])
            nc.sync.dma_start(bk_sb[:], bk[:])
            nc.sync.dma_start(bv_sb[:], bv[:])
            nc.sync.dma_start(ones_sb[:], ones[:])

            # persistent activations
            qcat = [pp.tile([128, L], F32R, tag=f"qcat{h}", name=f"qcat{h}")
                    for h in range(HPC)]
            kcat = [pp.tile([128, L], F32R, tag=f"kcat{h}", name=f"kcat{h}")
                    for h in range(HPC)]
            # v_aug per Lk chunk: (128, head, 128): even head slot s=0:
            # [ones | v], odd slot s=1: [v | ones]
            v_sb = [pp.tile([128, HPC, 128], F32R, tag=f"v{ck}", name=f"v{ck}")
                    for ck in range(NCHUNK)]
            yT = [pp.tile([128, L], F32R, tag=f"yT{j}", name=f"yT{j}")
                  for j in range(2)]

            # ---------------- phase 1: QKV ----------------
            with tc.tile_pool(name="xp", bufs=2) as xp, \
                 tc.tile_pool(name="stg", bufs=3) as stg, \
                 tc.tile_pool(name="p1ps", bufs=2, space="PSUM") as p1q, \
                 tc.tile_pool(name="p1psk", bufs=1, space="PSUM") as p1k, \
                 tc.tile_pool(name="p1psv", bufs=2, space="PSUM") as p1v:
                for ltb in range(NLTB):
                    ls = slice(ltb * LTB, (ltb + 1) * LTB)
                    xt_blk = xp.tile([128, EC, LTB], F32R, tag="x",
                                     name=f"xt{ltb}")
                    nc.sync.dma_start(xt_blk[:], xtT_v[:, :, ls])

                    # qt for head pairs
                    for j in range(2):
                        pq = p1q.tile([128, LTB], F32, tag="pq", name=f"pq{ltb}{j}")
                        for k in range(EC):
                            nc.tensor.matmul(
                                pq[:], wq_sb[:, k, j * 128:(j + 1) * 128],
                                xt_blk[:, k, :],
                                start=(k == 0), stop=(k == EC - 1),
                                skip_group_check=True)
                        nc.vector.tensor_scalar_add(
                            qcat[2 * j][0:64, ls], pq[0:64, :], bq_sb[0:64, j:j + 1])
                        qst = stg.tile([128, LTB], F32R, tag="qst",
                                       name=f"qst{ltb}{j}")
                        nc.vector.tensor_scalar_add(
                            qst[64:128, :], pq[64:128, :], bq_sb[64:128, j:j + 1])
                        nc.sync.dma_start(qcat[2 * j + 1][0:64, ls], qst[64:128, :])

                    # k stage A (xt part) — psums stay open
                    pk = []
                    for h in range(HPC):
                        pkh = p1k.tile([128, LTB], F32, tag=f"pk{h}",
                                       name=f"pk{ltb}{h}")
                        pk.append(pkh)
                        for k in range(EC):
                            nc.tensor.matmul(
                                pkh[:], wk_sb[:, k, h * 128:(h + 1) * 128],
                                xt_blk[:, k, :],
                                start=(k == 0), stop=False,
                                skip_group_check=True)

                    # v (natural layout) + bias, into v_aug slots
                    for vt in range(LTB // 128):
                        ck = ltb * (LTB // 128) + vt
                        pv = p1v.tile([128, HPC * D], F32, tag="pv",
                                      name=f"pv{ck}")
                        for k in range(EC):
                            nc.tensor.matmul(
                                pv[:], xt_blk[:, k, vt * 128:(vt + 1) * 128],
                                wv_sb[:, k, :],
                                start=(k == 0), stop=False,
                                skip_group_check=True)
                        nc.tensor.matmul(pv[:], ones_sb[0:1, :], bv_sb[:],
                                         start=False, stop=True,
                                         skip_group_check=True)
                        pv_v = pv.rearrange("p (h d) -> p h d", d=D)
                        # even heads (slot 0): v in cols 64:128
                        nc.vector.tensor_copy(v_sb[ck][:, 0::2, 64:128],
                                              pv_v[:, 0::2, :])
                        # odd heads (slot 1): v in cols 0:64
                        nc.vector.tensor_copy(v_sb[ck][:, 1::2, 0:64],
                                              pv_v[:, 1::2, :])
                        ones_v = ones.rearrange("p (s c) -> p s c", c=64)
                        nc.sync.dma_start(v_sb[ck][:, 0::2, 0:64], ones_v)
                        nc.sync.dma_start(v_sb[ck][:, 1::2, 64:128], ones_v)

                    xs_blk = xp.tile([128, EC, LTB], F32R, tag="x",
                                     name=f"xs{ltb}")
                    nc.sync.dma_start(xs_blk[:], xsT_v[:, :, ls])

                    # qs for head pairs
                    for j in range(2):
                        pq = p1q.tile([128, LTB], F32, tag="pq",
                                      name=f"pqs{ltb}{j}")
                        for k in range(EC):
                            nc.tensor.matmul(
                                pq[:], wqs_sb[:, k, j * 128:(j + 1) * 128],
                                xs_blk[:, k, :],
                                start=(k == 0), stop=(k == EC - 1),
                                skip_group_check=True)
                        qst = stg.tile([128, LTB], F32R, tag="qst",
                                       name=f"qsst{ltb}{j}")
                        nc.vector.tensor_scalar_add(
                            qst[0:64, :], pq[0:64, :], bqs_sb[0:64, j:j + 1])
                        nc.sync.dma_start(qcat[2 * j][64:128, ls], qst[0:64, :])
                        nc.vector.tensor_scalar_add(
                            qcat[2 * j + 1][64:128, ls], pq[64:128, :],
                            bqs_sb[64:128, j:j + 1])

                    # k stage B (xs part) + copy out
                    for h in range(HPC):
                        for k in range(EC):
                            nc.tensor.matmul(
                                pk[h][:], wk_sb[:, EC + k, h * 128:(h + 1) * 128],
                                xs_blk[:, k, :],
                                start=False, stop=(k == EC - 1),
                                skip_group_check=True)
                        nc.vector.tensor_scalar_add(
                            kcat[h][:, ls], pk[h][:], bk_sb[:, h:h + 1])

            # ---------------- phase 2: attention ----------------
            with tc.tile_pool(name="expp", bufs=4) as expp, \
                 tc.tile_pool(name="np2", bufs=2) as np2, \
                 tc.tile_pool(name="p2s", bufs=3, space="PSUM") as p2s, \
                 tc.tile_pool(name="p2y", bufs=2, space="PSUM") as p2y:
                for h in range(HPC):
                    j, s = h // 2, h % 2
                    sums_h = slice(0, 64) if s == 0 else slice(64, 128)
                    y_h = slice(64, 128) if s == 0 else slice(0, 64)
                    slot = slice(0, 64) if s == 0 else slice(64, 128)
                    for lq in range(4):
                        qs_ = slice(lq * 512, (lq + 1) * 512)
                        exps = []
                        for g in range(8):
                            ps = p2s.tile([128, 1024], F32, tag="ps",
                                          name=f"ps{h}{lq}{g}")
                            for half in range(2):
                                ck = 2 * g + half
                                nc.tensor.matmul(
                                    ps[:, half * 512:(half + 1) * 512],
                                    kcat[h][:, ck * 128:(ck + 1) * 128],
                                    qcat[h][:, qs_],
                                    start=True, stop=True,
                                    skip_group_check=True)
                            ex = expp.tile([128, 1024], F32R, tag="ex",
                                           name=f"ex{h}{lq}{g}")
                            nc.scalar.activation(ex[:], ps[:], AF.Exp,
                                                 scale=0.125)
                            exps.append(ex)
                        py = p2y.tile([128, 512], F32, tag="py",
                                      name=f"py{h}{lq}")
                        for g in range(8):
                            for half in range(2):
                                ck = 2 * g + half
                                nc.tensor.matmul(
                                    py[:], v_sb[ck][:, h, :],
                                    exps[g][:, half * 512:(half + 1) * 512],
                                    start=(ck == 0), stop=(ck == NCHUNK - 1),
                                    skip_group_check=True)
                        # normalize: recip of sums via ACT Ln -> Exp(-x),
                        # DMA-shift recip to the y partitions, TT multiply
                        # from PSUM into staging, DMA into the yT slot.
                        lnt = np2.tile([128, 512], F32, tag="lnt",
                                       name=f"ln{h}{lq}")
                        nc.scalar.activation(lnt[sums_h, :], py[sums_h, :],
                                             AF.Ln)
                        rec = np2.tile([128, 512], F32, tag="rec",
                                       name=f"rec{h}{lq}")
                        nc.scalar.activation(rec[sums_h, :], lnt[sums_h, :],
                                             AF.Exp, scale=-1.0)
                        rec2 = np2.tile([128, 512], F32, tag="rec2",
                                        name=f"rec2{h}{lq}")
                        nc.sync.dma_start(rec2[y_h, :], rec[sums_h, :])
                        yst = np2.tile([128, 512], F32R, tag="yst",
                                       name=f"yst{h}{lq}")
                        nc.vector.tensor_tensor(yst[y_h, :], py[y_h, :],
                                                rec2[y_h, :],
                                                mybir.AluOpType.mult)
                        nc.sync.dma_start(yT[j][slot, qs_], yst[y_h, :])

            # ---------------- phase 3: projection ----------------
            with tc.tile_pool(name="wc3", bufs=1) as wc3p, \
                 tc.tile_pool(name="outp", bufs=4) as outp, \
                 tc.tile_pool(name="p3o", bufs=4, space="PSUM") as p3o:
                wc_sb = wc3p.tile([128, 2, E], F32R)
                nc.sync.dma_start(wc_sb[:], wc[:])
                bc_sb = wc3p.tile([1, E], F32R)
                nc.sync.dma_start(bc_sb[:], bc[:])
                for lqt in range(L // 128):
                    lqs = slice(lqt * 128, (lqt + 1) * 128)
                    for nch in range(2):
                        ns = slice(nch * 512, (nch + 1) * 512)
                        po = p3o.tile([128, 512], F32, tag="po",
                                      name=f"po{lqt}{nch}")
                        nc.tensor.matmul(po[:], yT[0][:, lqs], wc_sb[:, 0, ns],
                                         start=True, stop=False,
                                         skip_group_check=True)
                        nc.tensor.matmul(po[:], yT[1][:, lqs], wc_sb[:, 1, ns],
                                         start=False, stop=False,
                                         skip_group_check=True)
                        nc.tensor.matmul(po[:], ones_sb[0:1, :], bc_sb[0:1, ns],
                                         start=False, stop=True,
                                         skip_group_check=True)
                        ot = outp.tile([128, 512], F32, tag="ot",
                                       name=f"ot{lqt}{nch}")
                        nc.vector.tensor_copy(ot[:], po[:])
                        nc.sync.dma_start(out[lqs, ns], ot[:])
    return nc


_NC_CACHE = None


def _get_nc():
    global _NC_CACHE
    if _NC_CACHE is None:
        nc = _build()
        _split_multi_waits(nc)
        _NC_CACHE = nc
    return _NC_CACHE


def _prep_core_inputs(core, xt, xs, Wt, bt, Ws, bs, Wc, bc, lam_ts, lam_st,
                      lam_ss):
    b, hg = core // HPC, core % HPC
    c0 = hg * HPC * D  # 256*hg
    lts, lst, lss = float(lam_ts[0]), float(lam_st[0]), float(lam_ss[0])

    wq_full = Wt[:, c0:c0 + HPC * D]                     # (E, 256) qt
    wqs_full = Ws[:, c0:c0 + HPC * D]                    # (E, 256) qs
    wv_full = Wt[:, 2 * E + c0:2 * E + c0 + HPC * D]     # (E, 256)
    ktw = Wt[:, E + c0:E + c0 + HPC * D]                 # (E, 256)
    ksw = Ws[:, E + c0:E + c0 + HPC * D]                 # (E, 256)

    # wk_full (2E, 512): per head h cols h*128: [k1 | k2]
    wk_full = np.zeros((2 * E, 2 * HPC * D), np.float32)
    for h in range(HPC):
        hs = slice(h * D, (h + 1) * D)
        wk_full[:E, h * 128:h * 128 + D] = ktw[:, hs]
        wk_full[:E, h * 128 + D:(h + 1) * 128] = lst * ktw[:, hs]
        wk_full[E:, h * 128:h * 128 + D] = lts * ksw[:, hs]
        wk_full[E:, h * 128 + D:(h + 1) * 128] = lss * ksw[:, hs]

    def chunked(a, nk):
        return np.ascontiguousarray(
            a.reshape(nk, 128, a.shape[1]).transpose(1, 0, 2))

    btq = bt[c0:c0 + HPC * D]
    bsq = bs[c0:c0 + HPC * D]
    btk = bt[E + c0:E + c0 + HPC * D]
    bsk = bs[E + c0:E + c0 + HPC * D]
    bq_arr = np.zeros((128, 2), np.float32)
    bqs_arr = np.zeros((128, 2), np.float32)
    bk_arr = np.zeros((128, HPC), np.float32)
    for j in range(2):
        bq_arr[0:64, j] = btq[(2 * j) * D:(2 * j + 1) * D]
        bq_arr[64:128, j] = btq[(2 * j + 1) * D:(2 * j + 2) * D]
        bqs_arr[0:64, j] = bsq[(2 * j) * D:(2 * j + 1) * D]
        bqs_arr[64:128, j] = bsq[(2 * j + 1) * D:(2 * j + 2) * D]
    for h in range(HPC):
        hs = slice(h * D, (h + 1) * D)
        bk_arr[0:64, h] = btk[hs] + lts * bsk[hs]
        bk_arr[64:128, h] = lst * btk[hs] + lss * bsk[hs]

    return {
        "xtT": np.ascontiguousarray(xt[b].T),
        "xsT": np.ascontiguousarray(xs[b].T),
        "wq": chunked(wq_full, EC),
        "wqs": chunked(wqs_full, EC),
        "wk": chunked(wk_full, 2 * EC),
        "wv": chunked(wv_full, EC),
        "wc": chunked(Wc[c0:c0 + HPC * D, :], 2),
        "bq": bq_arr,
        "bqs": bqs_arr,
        "bk": bk_arr,
        "bv": bt[2 * E + c0:2 * E + c0 + HPC * D].reshape(1, HPC * D).copy(),
        "bc": (bc if hg == 0 else np.zeros_like(bc)).reshape(1, E).copy(),
        "ones": np.ones((128, 128), np.float32),
    }


def kernel(**inputs):
    xt = np.asarray(inputs["xt"], np.float32)
    xs = np.asarray(inputs["xs"], np.float32)
    args = dict(
        xt=xt, xs=xs,
        Wt=np.asarray(inputs["Wt"], np.float32),
        bt=np.asarray(inputs["bt"], np.float32),
        Ws=np.asarray(inputs["Ws"], np.float32),
        bs=np.asarray(inputs["bs"], np.float32),
        Wc=np.asarray(inputs["Wc"], np.float32),
        bc=np.asarray(inputs["bc"], np.float32),
        lam_ts=np.asarray(inputs["lam_ts"], np.float32),
        lam_st=np.asarray(inputs["lam_st"], np.float32),
        lam_ss=np.asarray(inputs["lam_ss"], np.float32),
    )
    in_maps = [_prep_core_inputs(c, **args) for c in range(NCORES)]
    nc = _get_nc()
    res = run_bass_kernel_spmd(nc, in_maps, list(range(NCORES)))
    out = np.zeros((B, L, E), np.float32)
    for c in range(NCORES):
        out[c // HPC] += res.results[c]["out"]
    return out


# revision 6
# speedup vs baseline: 1.1353x; 1.1353x over previous
"""Disentangled spatial attention TRN2 kernel (8 NeuronCores).

Sharding: 8 cores = 2 batches x 4 head-groups (4 heads each).
Per core, transposed-activation layout:
  qcat[h] (128, L):  rows 0:64 qt_h, rows 64:128 qs_h
  kcat[h] (128, L):  rows 0:64 k1_h = kt + lam_ts*ks,
                     rows 64:128 k2_h = lam_st*kt + lam_ss*ks
  scores^T chunk = kcat_chunk.T @ qcat  (both reference score einsums
  fused into one K=128 matmul; lam_* folded into weight shards on host)
  softmax row-sums ride along the PV matmul as 64 replicated "ones"
  columns of the v operand; normalization happens on the way into the
  transposed y layout that feeds the output projection.
Phase-1 matmuls run in float32r; attention + projection operands are
bf16 (fp32 PSUM accumulation).  v/c biases are folded in on the host
(exact: softmax rows sum to 1), qkv biases are added on device.
"""
import numpy as np
import ml_dtypes
import concourse.bass as bass
import concourse.mybir as mybir
import concourse.tile as tile
from concourse.bass_utils import run_bass_kernel_spmd

F32 = mybir.dt.float32
F32R = mybir.dt.float32r
BF16 = mybir.dt.bfloat16
AF = mybir.ActivationFunctionType

B, L, E, H, D = 2, 2048, 1024, 16, 64
HPC = 4          # heads per core
NCORES = 8
LTB = 512        # L block for phase 1
NLTB = L // LTB  # 4
NCHUNK = L // 128  # 16 Lk chunks
EC = E // 128    # 8 E chunks


def _split_multi_waits(nc, max_waits=1):
    """walrus codegen allows only one sync wait per instruction; move extra
    waits onto standalone same-engine NoOps placed just before."""
    n_split = 0
    for f in nc.m.functions:
        for blk in f.blocks:
            insts = list(blk.instructions)
            out = []
            changed = False
            for inst in insts:
                si = inst.sync_info
                waits = list(si.on_wait) if si is not None and si.on_wait else []
                if len(waits) > max_waits:
                    keep = waits[-max_waits:]
                    extra = waits[:-max_waits]
                    for w in extra:
                        nop = mybir.InstNoOp(
                            name=f"{inst.name}-wsplit{n_split}",
                            engine=inst.engine,
                            ins=[], outs=[],
                            sync_info=mybir.SyncInfo(on_wait=[w], on_update=[]),
                        )
                        out.append(nop)
                        n_split += 1
                    inst.sync_info = mybir.SyncInfo(
                        on_wait=keep,
                        on_update=list(si.on_update) if si.on_update else [],
                    )
                    changed = True
                out.append(inst)
            if changed:
                blk.instructions = out
    return n_split


def _build():
    nc = bass.Bass()
    xtT = nc.declare_dram_parameter("xtT", [E, L], F32R, isOutput=False)
    xsT = nc.declare_dram_parameter("xsT", [E, L], F32R, isOutput=False)
    wq = nc.declare_dram_parameter("wq", [128, EC, HPC * D], F32R, isOutput=False)
    wqs = nc.declare_dram_parameter("wqs", [128, EC, HPC * D], F32R, isOutput=False)
    wk = nc.declare_dram_parameter("wk", [128, 2 * EC, 2 * HPC * D], F32R, isOutput=False)
    wv = nc.declare_dram_parameter("wv", [128, EC, HPC * D], F32R, isOutput=False)
    wc = nc.declare_dram_parameter("wc", [128, 2, E], BF16, isOutput=False)
    bq = nc.declare_dram_parameter("bq", [128, 2], F32, isOutput=False)
    bqs = nc.declare_dram_parameter("bqs", [128, 2], F32, isOutput=False)
    bk = nc.declare_dram_parameter("bk", [128, HPC], F32, isOutput=False)
    ones = nc.declare_dram_parameter("ones", [128, NCHUNK, 2, 64], BF16,
                                     isOutput=False)
    out = nc.declare_dram_parameter("out", [L, E], F32, isOutput=True)

    xtT_v = xtT.rearrange("(k p) l -> p k l", p=128)   # (128, 8, L)
    xsT_v = xsT.rearrange("(k p) l -> p k l", p=128)

    with tile.TileContext(nc) as tc:
        with tc.tile_pool(name="wpool", bufs=1) as wpool, \
             tc.tile_pool(name="persist", bufs=1) as pp:
            # persistent activations (bf16)
            qcat = [pp.tile([128, L], BF16, tag=f"qcat{h}", name=f"qcat{h}")
                    for h in range(HPC)]
            kcat = [pp.tile([128, L], BF16, tag=f"kcat{h}", name=f"kcat{h}")
                    for h in range(HPC)]
            # v_aug: (128, chunk, head, 128); head slot s=0: [ones | v],
            # s=1: [v | ones]
            v_sb = pp.tile([128, NCHUNK, HPC, 128], BF16, name="v_sb")
            yT = [pp.tile([128, L], BF16, tag=f"yT{j}", name=f"yT{j}")
                  for j in range(2)]
            # staging for partition-shifted qcat halves: per pair j,
            # rows 64:128 <- qt_{2j+1}, rows 0:64 <- qs_{2j}
            qstg = [pp.tile([128, L], BF16, tag=f"qstg{j}", name=f"qstg{j}")
                    for j in range(2)]

            wq_sb = wpool.tile([128, EC, HPC * D], F32R)
            wqs_sb = wpool.tile([128, EC, HPC * D], F32R)
            wk_sb = wpool.tile([128, 2 * EC, 2 * HPC * D], F32R)
            wv_sb = wpool.tile([128, EC, HPC * D], F32R)
            bq_sb = wpool.tile([128, 2], F32)
            bqs_sb = wpool.tile([128, 2], F32)
            bk_sb = wpool.tile([128, HPC], F32)

            # ---------------- phase 1: QKV ----------------
            with tc.tile_pool(name="xp", bufs=2) as xp, \
                 tc.tile_pool(name="p1ps", bufs=2, space="PSUM") as p1q, \
                 tc.tile_pool(name="p1psk", bufs=1, space="PSUM") as p1k, \
                 tc.tile_pool(name="p1psv", bufs=2, space="PSUM") as p1v:
                first = True
                for ltb in range(NLTB):
                    ls = slice(ltb * LTB, (ltb + 1) * LTB)
                    xt_blk = xp.tile([128, EC, LTB], F32R, tag="x",
                                     name=f"xt{ltb}")
                    nc.sync.dma_start(xt_blk[:], xtT_v[:, :, ls])
                    if first:
                        # x block first, then weights in use order
                        nc.sync.dma_start(wq_sb[:], wq[:])
                        nc.sync.dma_start(bq_sb[:], bq[:])
                        nc.sync.dma_start(wk_sb[:], wk[:])
                        nc.sync.dma_start(wv_sb[:], wv[:])
                        nc.sync.dma_start(wqs_sb[:], wqs[:])
                        nc.sync.dma_start(bqs_sb[:], bqs[:])
                        nc.sync.dma_start(bk_sb[:], bk[:])
                        nc.sync.dma_start(v_sb[:, :, 0::2, 0:64], ones[:])
                        nc.sync.dma_start(v_sb[:, :, 1::2, 64:128], ones[:])
                        first = False

                    # qt for head pairs
                    for j in range(2):
                        pq = p1q.tile([128, LTB], F32, tag="pq", name=f"pq{ltb}{j}")
                        for k in range(EC):
                            nc.tensor.matmul(
                                pq[:], wq_sb[:, k, j * 128:(j + 1) * 128],
                                xt_blk[:, k, :],
                                start=(k == 0), stop=(k == EC - 1),
                                skip_group_check=True)
                        nc.vector.tensor_scalar_add(
                            qcat[2 * j][0:64, ls], pq[0:64, :], bq_sb[0:64, j:j + 1])
                        nc.vector.tensor_scalar_add(
                            qstg[j][64:128, ls], pq[64:128, :], bq_sb[64:128, j:j + 1])

                    # k stage A (xt part) — psums stay open
                    pk = []
                    for h in range(HPC):
                        pkh = p1k.tile([128, LTB], F32, tag=f"pk{h}",
                                       name=f"pk{ltb}{h}")
                        pk.append(pkh)
                        for k in range(EC):
                            nc.tensor.matmul(
                                pkh[:], wk_sb[:, k, h * 128:(h + 1) * 128],
                                xt_blk[:, k, :],
                                start=(k == 0), stop=False,
                                skip_group_check=True)

                    # v (natural layout) into v_aug slots
                    for vt in range(LTB // 128):
                        ck = ltb * (LTB // 128) + vt
                        pv = p1v.tile([128, HPC * D], F32, tag="pv",
                                      name=f"pv{ck}")
                        for k in range(EC):
                            nc.tensor.matmul(
                                pv[:], xt_blk[:, k, vt * 128:(vt + 1) * 128],
                                wv_sb[:, k, :],
                                start=(k == 0), stop=(k == EC - 1),
                                skip_group_check=True)
                        pv_v = pv.rearrange("p (h d) -> p h d", d=D)
                        # even heads (slot 0): v in cols 64:128
                        nc.vector.tensor_copy(v_sb[:, ck, 0::2, 64:128],
                                              pv_v[:, 0::2, :])
                        # odd heads (slot 1): v in cols 0:64
                        nc.vector.tensor_copy(v_sb[:, ck, 1::2, 0:64],
                                              pv_v[:, 1::2, :])

                    xs_blk = xp.tile([128, EC, LTB], F32R, tag="x",
                                     name=f"xs{ltb}")
                    nc.sync.dma_start(xs_blk[:], xsT_v[:, :, ls])

                    # qs for head pairs
                    for j in range(2):
                        pq = p1q.tile([128, LTB], F32, tag="pq",
                                      name=f"pqs{ltb}{j}")
                        for k in range(EC):
                            nc.tensor.matmul(
                                pq[:], wqs_sb[:, k, j * 128:(j + 1) * 128],
                                xs_blk[:, k, :],
                                start=(k == 0), stop=(k == EC - 1),
                                skip_group_check=True)
                        nc.vector.tensor_scalar_add(
                            qstg[j][0:64, ls], pq[0:64, :], bqs_sb[0:64, j:j + 1])
                        nc.vector.tensor_scalar_add(
                            qcat[2 * j + 1][64:128, ls], pq[64:128, :],
                            bqs_sb[64:128, j:j + 1])

                    # k stage B (xs part) + copy out
                    for h in range(HPC):
                        for k in range(EC):
                            nc.tensor.matmul(
                                pk[h][:], wk_sb[:, EC + k, h * 128:(h + 1) * 128],
                                xs_blk[:, k, :],
                                start=False, stop=(k == EC - 1),
                                skip_group_check=True)
                        nc.vector.tensor_scalar_add(
                            kcat[h][:, ls], pk[h][:], bk_sb[:, h:h + 1])

                # resolve staged qcat halves (partition shifts via DMA)
                for j in range(2):
                    nc.sync.dma_start(qcat[2 * j + 1][0:64, :], qstg[j][64:128, :])
                    nc.sync.dma_start(qcat[2 * j][64:128, :], qstg[j][0:64, :])

            # ---------------- phase 2: attention ----------------
            with tc.tile_pool(name="expp", bufs=6) as expp, \
                 tc.tile_pool(name="np2", bufs=2) as np2, \
                 tc.tile_pool(name="p2s", bufs=3, space="PSUM") as p2s, \
                 tc.tile_pool(name="p2y", bufs=2, space="PSUM") as p2y:
                for h in range(HPC):
                    j, s = h // 2, h % 2
                    sums_h = slice(0, 64) if s == 0 else slice(64, 128)
                    y_h = slice(64, 128) if s == 0 else slice(0, 64)
                    slot = slice(0, 64) if s == 0 else slice(64, 128)
                    for lq in range(4):
                        qs_ = slice(lq * 512, (lq + 1) * 512)
                        py = p2y.tile([128, 512], F32, tag="py",
                                      name=f"py{h}{lq}")
                        for g in range(8):
                            ps = p2s.tile([128, 1024], F32, tag="ps",
                                          name=f"ps{h}{lq}{g}")
                            for half in range(2):
                                ck = 2 * g + half
                                nc.tensor.matmul(
                                    ps[:, half * 512:(half + 1) * 512],
                                    kcat[h][:, ck * 128:(ck + 1) * 128],
                                    qcat[h][:, qs_],
                                    start=True, stop=True,
                                    skip_group_check=True)
                            ex = expp.tile([128, 1024], BF16, tag="ex",
                                           name=f"ex{h}{lq}{g}")
                            nc.scalar.activation(ex[:], ps[:], AF.Exp,
                                                 scale=0.125)
                            for half in range(2):
                                ck = 2 * g + half
                                nc.tensor.matmul(
                                    py[:], v_sb[:, ck, h, :],
                                    ex[:, half * 512:(half + 1) * 512],
                                    start=(ck == 0), stop=(ck == NCHUNK - 1),
                                    skip_group_check=True)
                        # normalize: recip = Exp(-Ln(sums)); DMA-shift recip
                        # to the y partitions; TT multiply PSUM -> staging;
                        # DMA staging into the yT slot.
                        lnt = np2.tile([128, 512], F32, tag="lnt",
                                       name=f"ln{h}{lq}")
                        nc.scalar.activation(lnt[sums_h, :], py[sums_h, :],
                                             AF.Ln)
                        rec = np2.tile([128, 512], F32, tag="rec",
                                       name=f"rec{h}{lq}")
                        nc.scalar.activation(rec[sums_h, :], lnt[sums_h, :],
                                             AF.Exp, scale=-1.0)
                        rec2 = np2.tile([128, 512], F32, tag="rec2",
                                        name=f"rec2{h}{lq}")
                        nc.sync.dma_start(rec2[y_h, :], rec[sums_h, :])
                        yst = np2.tile([128, 512], BF16, tag="yst",
                                       name=f"yst{h}{lq}")
                        nc.vector.tensor_tensor(yst[y_h, :], py[y_h, :],
                                                rec2[y_h, :],
                                                mybir.AluOpType.mult)
                        nc.sync.dma_start(yT[j][slot, qs_], yst[y_h, :])

            # ---------------- phase 3: projection ----------------
            with tc.tile_pool(name="wc3", bufs=1) as wc3p, \
                 tc.tile_pool(name="outp", bufs=3) as outp, \
                 tc.tile_pool(name="p3o", bufs=4, space="PSUM") as p3o:
                wc_sb = wc3p.tile([128, 2, E], BF16)
                nc.sync.dma_start(wc_sb[:], wc[:])
                for lqt in range(L // 128):
                    lqs = slice(lqt * 128, (lqt + 1) * 128)
                    ot = outp.tile([128, E], F32, tag="ot", name=f"ot{lqt}")
                    for nch in range(2):
                        ns = slice(nch * 512, (nch + 1) * 512)
                        po = p3o.tile([128, 512], F32, tag="po",
                                      name=f"po{lqt}{nch}")
                        nc.tensor.matmul(po[:], yT[0][:, lqs], wc_sb[:, 0, ns],
                                         start=True, stop=False,
                                         skip_group_check=True)
                        nc.tensor.matmul(po[:], yT[1][:, lqs], wc_sb[:, 1, ns],
                                         start=False, stop=True,
                                         skip_group_check=True)
                        if nch == 0:
                            nc.vector.tensor_copy(ot[:, ns], po[:])
                        else:
                            nc.scalar.copy(ot[:, ns], po[:])
                    nc.sync.dma_start(out[lqs, :], ot[:])
    return nc


_NC_CACHE = None


def _get_nc():
    global _NC_CACHE
    if _NC_CACHE is None:
        nc = _build()
        _split_multi_waits(nc)
        _NC_CACHE = nc
    return _NC_CACHE


def _prep_core_inputs(core, xt, xs, Wt, bt, Ws, bs, Wc, bc, lam_ts, lam_st,
                      lam_ss):
    b, hg = core // HPC, core % HPC
    c0 = hg * HPC * D  # 256*hg
    lts, lst, lss = float(lam_ts[0]), float(lam_st[0]), float(lam_ss[0])

    wq_full = Wt[:, c0:c0 + HPC * D]                     # (E, 256) qt
    wqs_full = Ws[:, c0:c0 + HPC * D]                    # (E, 256) qs
    wv_full = Wt[:, 2 * E + c0:2 * E + c0 + HPC * D]     # (E, 256)
    ktw = Wt[:, E + c0:E + c0 + HPC * D]                 # (E, 256)
    ksw = Ws[:, E + c0:E + c0 + HPC * D]                 # (E, 256)

    wk_full = np.zeros((2 * E, 2 * HPC * D), np.float32)
    for h in range(HPC):
        hs = slice(h * D, (h + 1) * D)
        wk_full[:E, h * 128:h * 128 + D] = ktw[:, hs]
        wk_full[:E, h * 128 + D:(h + 1) * 128] = lst * ktw[:, hs]
        wk_full[E:, h * 128:h * 128 + D] = lts * ksw[:, hs]
        wk_full[E:, h * 128 + D:(h + 1) * 128] = lss * ksw[:, hs]

    def chunked(a, nk, dtype=np.float32):
        return np.ascontiguousarray(
            a.reshape(nk, 128, a.shape[1]).transpose(1, 0, 2)).astype(dtype)

    btq = bt[c0:c0 + HPC * D]
    bsq = bs[c0:c0 + HPC * D]
    btk = bt[E + c0:E + c0 + HPC * D]
    bsk = bs[E + c0:E + c0 + HPC * D]
    bq_arr = np.zeros((128, 2), np.float32)
    bqs_arr = np.zeros((128, 2), np.float32)
    bk_arr = np.zeros((128, HPC), np.float32)
    for j in range(2):
        bq_arr[0:64, j] = btq[(2 * j) * D:(2 * j + 1) * D]
        bq_arr[64:128, j] = btq[(2 * j + 1) * D:(2 * j + 2) * D]
        bqs_arr[0:64, j] = bsq[(2 * j) * D:(2 * j + 1) * D]
        bqs_arr[64:128, j] = bsq[(2 * j + 1) * D:(2 * j + 2) * D]
    for h in range(HPC):
        hs = slice(h * D, (h + 1) * D)
        bk_arr[0:64, h] = btk[hs] + lts * bsk[hs]
        bk_arr[64:128, h] = lst * btk[hs] + lss * bsk[hs]

    return {
        "xtT": np.ascontiguousarray(xt[b].T),
        "xsT": np.ascontiguousarray(xs[b].T),
        "wq": chunked(wq_full, EC),
        "wqs": chunked(wqs_full, EC),
        "wk": chunked(wk_full, 2 * EC),
        "wv": chunked(wv_full, EC),
        "wc": chunked(Wc[c0:c0 + HPC * D, :], 2, ml_dtypes.bfloat16),
        "bq": bq_arr,
        "bqs": bqs_arr,
        "bk": bk_arr,
        "ones": np.ones((128, NCHUNK, 2, 64), ml_dtypes.bfloat16),
    }


def kernel(**inputs):
    xt = np.asarray(inputs["xt"], np.float32)
    xs = np.asarray(inputs["xs"], np.float32)
    Wc = np.asarray(inputs["Wc"], np.float32)
    bt = np.asarray(inputs["bt"], np.float32)
    bc = np.asarray(inputs["bc"], np.float32)
    args = dict(
        xt=xt, xs=xs,
        Wt=np.asarray(inputs["Wt"], np.float32),
        bt=bt,
        Ws=np.asarray(inputs["Ws"], np.float32),
        bs=np.asarray(inputs["bs"], np.float32),
        Wc=Wc, bc=bc,
        lam_ts=np.asarray(inputs["lam_ts"], np.float32),
        lam_st=np.asarray(inputs["lam_st"], np.float32),
        lam_ss=np.asarray(inputs["lam_ss"], np.float32),
    )
    in_maps = [_prep_core_inputs(c, **args) for c in range(NCORES)]
    nc = _get_nc()
    res = run_bass_kernel_spmd(nc, in_maps, list(range(NCORES)))
    out = np.zeros((B, L, E), np.float32)
    for c in range(NCORES):
        out[c // HPC] += res.results[c]["out"]
    # v-bias and c-bias folded in on the host: softmax rows sum to one, so
    # the v bias contributes bv @ Wc (a constant row) to every position.
    out += bt[2 * E:] @ Wc + bc
    return out
